# revision 4
# baseline (speedup 1.0000x reference)
"""Trainium2 Bass kernel for nn_DiscriminatorBlock (StyleGAN2 discriminator
block), parity-class ("mod-2 x mod-2") layout.

Per core (batch 8 over 8 cores): all tensors live in SBUF as
  T[32*g + c, jy, jx]  with g = 2*(y%2) + (x%2), jy=y>>1, jx=x>>1
so a 3x3 (or fused 6x6/4x4 stride-2) conv becomes a small set of dense
matmuls: for each "view" (a jy/jx shift of the rhs), every 32x32 block
(g, m) of the 128x128 weight matrix carries one conv tap. The dense
view (0,0) carries 16 taps in one full-array matmul; edge/corner views
carry the wrap-around taps and are emitted as narrow tile-positioned
matmuls that can run concurrently on disjoint PE sub-arrays.

conv0 (3x3 SAME): 9 matmuls per 2-row group of h-halfres rows.
conv1 (FIR-blurred 3x3 stride 2 == fused 6x6 stride 2): 20 matmuls per
8-out-row group, all with fully dense K=128.
res (FIR 4x4 stride 2 x w_res): 20 matmuls, mostly dense.
FIR blurs + equalized-lr scales + sqrt(2)/sqrt(0.5) gains folded into
the fp16 weights on the host; fp32 PSUM accumulate; Prelu on ACT.
"""
import sys
import os
sys.path.insert(0, '/opt/trn_rl_repo')
import numpy as np

H, W, C, F = 512, 512, 32, 32
N_CORES = 8
JX = W // 2            # 256 jx values per row
RP = JX + 32           # row pitch with zero guard cols
GL = 32                # leading guard cols
SO = 64                # out1 rows per tile
RB = 8                 # jy rows per load chunk

SQ2 = float(np.sqrt(2.0))
SQH = float(np.sqrt(0.5))

SPLIT_X = True         # split x-edge matmuls for 4-way col concurrency


def _mk_specs():
    """Per-conv ordered matmul specs: view shift (vy,vx), k/m 32-block
    ranges, start/stop flags, weight-tile column offset."""
    specs = {"c0": [], "c1": [], "rs": []}
    off = [0]

    def add(conv, vy, vx, k0, k1, m0, m1, start=False, stop=False):
        w = 32 * (m1 - m0)
        specs[conv].append(dict(vy=vy, vx=vx, k0=k0, k1=k1, m0=m0, m1=m1,
                                start=start, stop=stop, off=off[0], wid=w))
        off[0] += w

    # ---- c0: 3x3 SAME, views (ey,ex) in {-1,0,1}^2 ----
    # 4 slots: full | edge-y pair | x-edge quad | corner quad
    add("c0", 0, 0, 0, 4, 0, 4, start=True)
    add("c0", -1, 0, 2, 4, 0, 2)
    add("c0", 1, 0, 0, 2, 2, 4)
    add("c0", 0, -1, 0, 4, 0, 1)
    add("c0", 0, 1, 0, 3, 1, 2)
    add("c0", 0, -1, 0, 4, 2, 3)
    add("c0", 0, 1, 0, 3, 3, 4)
    add("c0", -1, -1, 3, 4, 0, 1, stop=True)
    add("c0", -1, 1, 2, 3, 1, 2, stop=True)
    add("c0", 1, -1, 1, 2, 2, 3, stop=True)
    add("c0", 1, 1, 0, 1, 3, 4, stop=True)

    # ---- c1: fused 6x6 stride 2, views (ay,ax) in {-1,0,1,2}^2 ----
    add("c1", 0, 0, 0, 4, 0, 4, start=True)
    add("c1", 0, 1, 0, 4, 0, 4)
    add("c1", 1, 0, 0, 4, 0, 4)
    add("c1", 1, 1, 0, 4, 0, 4)
    add("c1", -1, 0, 0, 4, 0, 2)
    add("c1", 2, 0, 0, 4, 2, 4)
    add("c1", -1, 1, 0, 4, 0, 2)
    add("c1", 2, 1, 0, 4, 2, 4)
    if SPLIT_X:
        for vy in (0, 1):
            add("c1", vy, -1, 0, 4, 0, 1)
            add("c1", vy, -1, 0, 4, 2, 3)
            add("c1", vy, 2, 0, 4, 1, 2)
            add("c1", vy, 2, 0, 4, 3, 4)
    else:
        for vy in (0, 1):
            add("c1", vy, -1, 0, 4, 0, 3)
            add("c1", vy, 2, 0, 4, 1, 4)
    add("c1", -1, -1, 0, 4, 0, 1, stop=True)
    add("c1", -1, 2, 0, 4, 1, 2, stop=True)
    add("c1", 2, -1, 0, 4, 2, 3, stop=True)
    add("c1", 2, 2, 0, 4, 3, 4, stop=True)

    # ---- rs: fused 4x4 stride 2 ----
    add("rs", 0, 0, 0, 4, 0, 4, start=True)
    add("rs", 0, 1, 0, 4, 0, 4)
    add("rs", 1, 0, 0, 4, 0, 4)
    add("rs", 1, 1, 0, 4, 0, 4)
    add("rs", -1, 0, 2, 4, 0, 2)
    add("rs", 2, 0, 0, 2, 2, 4)
    add("rs", -1, 1, 2, 4, 0, 2)
    add("rs", 2, 1, 0, 2, 2, 4)
    if SPLIT_X:
        for vy in (0, 1):
            add("rs", vy, -1, 0, 4, 0, 1)
            add("rs", vy, -1, 0, 4, 2, 3)
            add("rs", vy, 2, 0, 3, 1, 2)
            add("rs", vy, 2, 0, 3, 3, 4)
    else:
        for vy in (0, 1):
            add("rs", vy, -1, 0, 4, 0, 3)
            add("rs", vy, 2, 0, 3, 1, 4)
    add("rs", -1, -1, 3, 4, 0, 1, stop=True)
    add("rs", -1, 2, 2, 3, 1, 2, stop=True)
    add("rs", 2, -1, 1, 2, 2, 3, stop=True)
    add("rs", 2, 2, 0, 1, 3, 4, stop=True)

    return specs, off[0]


_SPECS, _WCOLS = _mk_specs()


def _tap_c0(vy, vx, g, m):
    py, px = g >> 1, g & 1
    my, mx = m >> 1, m & 1
    dy = 2 * vy + py - my
    dx = 2 * vx + px - mx
    if -1 <= dy <= 1 and -1 <= dx <= 1:
        return (dy + 1, dx + 1)
    return None


def _tap_c1(vy, vx, g, m):
    py, px = g >> 1, g & 1
    qy, qx = m >> 1, m & 1
    ey, ex = vy - qy + 1, vx - qx + 1
    if 0 <= ey <= 2 and 0 <= ex <= 2:
        return (2 * ey + py, 2 * ex + px)
    return None


def _rs1(v, p, q):
    if p == 1:
        if v == q - 1:
            return 0
        if v == q:
            return 2
    else:
        if v == q:
            return 1
        if v == q + 1:
            return 3
    return None


def _tap_rs(vy, vx, g, m):
    py, px = g >> 1, g & 1
    qy, qx = m >> 1, m & 1
    dy = _rs1(vy, py, qy)
    dx = _rs1(vx, px, qx)
    if dy is None or dx is None:
        return None
    return (dy, dx)


_TAP = {"c0": _tap_c0, "c1": _tap_c1, "rs": _tap_rs}
_NTAP = {"c0": 3, "c1": 6, "rs": 4}


def _selftest():
    """Every (out-class, tap) pair must be covered exactly once."""
    for conv in ("c0", "c1", "rs"):
        n = _NTAP[conv]
        cnt = np.zeros((4, n, n), np.int32)
        for sp in _SPECS[conv]:
            for g in range(sp["k0"], sp["k1"]):
                for m in range(sp["m0"], sp["m1"]):
                    t = _TAP[conv](sp["vy"], sp["vx"], g, m)
                    if t is not None:
                        cnt[m, t[0], t[1]] += 1
        assert (cnt == 1).all(), (conv, cnt)
    return True


def _pack_host(w0, b0, w1, b1, w_res):
    bk = np.array([1.0, 3.0, 3.0, 1.0]) / 8.0
    W0e = w0.astype(np.float64) * (1.0 / np.sqrt(9 * 32))
    W1e = w1.astype(np.float64) * (1.0 / np.sqrt(9 * 32))
    WRe = w_res.astype(np.float64) * (1.0 / np.sqrt(1 * 32))
    K6 = np.zeros((6, 6, 32, 32))
    for r in range(3):
        for s in range(3):
            for a in range(4):
                for c in range(4):
                    K6[r + a, s + c] += bk[a] * bk[c] * W1e[r, s]
    K6 *= SQ2
    K4 = np.einsum('a,c,ij->acij', bk, bk, WRe[0, 0]) * SQH
    KK = {"c0": W0e, "c1": K6, "rs": K4}

    wts = np.zeros((128, _WCOLS), np.float16)
    for conv in ("c0", "c1", "rs"):
        for sp in _SPECS[conv]:
            for g in range(sp["k0"], sp["k1"]):
                for m in range(sp["m0"], sp["m1"]):
                    t = _TAP[conv](sp["vy"], sp["vx"], g, m)
                    if t is None:
                        continue
                    wts[32 * g:32 * g + 32,
                        sp["off"] + 32 * (m - sp["m0"]):
                        sp["off"] + 32 * (m - sp["m0"]) + 32] = \
                        KK[conv][t[0], t[1]].astype(np.float16)
    biases = np.zeros((128, 2), np.float32)
    biases[:, 0] = np.tile(b0.astype(np.float32).reshape(-1), 4)
    biases[:, 1] = np.tile(b1.astype(np.float32).reshape(-1), 4)
    return wts, biases


def _build():
    import concourse.mybir as mybir
    from concourse import bacc
    from concourse.tile import TileContext

    F32 = mybir.dt.float32
    F16 = mybir.dt.float16
    ACTF = mybir.ActivationFunctionType

    HO = H // 2            # 256 out1 rows
    T = HO // SO           # tiles
    NG0 = SO // 2 + 1      # c0 groups per tile (2 h-halfres rows each)
    NGQ = SO // 8          # c1/rs groups per tile (8 out rows each)
    LXJ = SO + 4           # x slab jy rows (68 used)
    LHJ = SO + 4           # h slab rows (66 used)
    NJY = H // 2           # 256 valid jy rows

    nc = bacc.Bacc('TRN2', target_bir_lowering=False)
    # x pre-relayouted on host: [32*g+c, jy, jx] fp16, g=2*(y%2)+(x%2)
    x_t = nc.dram_tensor("x", [128, NJY, JX], F16, kind="ExternalInput")
    w_t = nc.dram_tensor("wts", [128, _WCOLS], F16, kind="ExternalInput")
    b_t = nc.dram_tensor("biases", [128, 2], F32, kind="ExternalInput")
    # out in class layout: [32*m+f, jy'', jx''], un-shuffled on host
    o_t = nc.dram_tensor("out", [128, HO // 2, HO // 2], F32,
                         kind="ExternalOutput")

    TSX = GL + LXJ * RP + 8
    TSH = GL + LHJ * RP + 8

    with TileContext(nc) as tc:
        with tc.tile_pool(name="const", bufs=1) as cpool, \
             tc.tile_pool(name="slab", bufs=2) as spool, \
             tc.tile_pool(name="hslab", bufs=1) as hpool, \
             tc.tile_pool(name="stage", bufs=3) as gpool, \
             tc.tile_pool(name="psum0", bufs=4, space="PSUM") as ppool0, \
             tc.tile_pool(name="psum", bufs=2, space="PSUM") as ppool:

            wtile = cpool.tile([128, _WCOLS], F16, tag="wts")
            nc.sync.dma_start(out=wtile[:, :], in_=w_t[:, :])
            btile = cpool.tile([128, 2], F32, tag="bias")
            nc.sync.dma_start(out=btile[:, :], in_=b_t[:, :])

            slabs = {}

            def emit_load(t):
                jy0 = SO * t - 2
                X4 = spool.tile([128, TSX], F16, tag="X4")
                Xv = {d: X4[:, GL + d:GL + d + LXJ * RP].rearrange(
                    "p (s u) -> p s u", u=RP) for d in (-1, 0, 1, 2)}
                slabs[t] = (X4, Xv)

                sv0 = max(0, -jy0)
                sv1 = min(LXJ, NJY - jy0)
                # chunked (small first chunk) so the first conv0 groups can
                # start early; on the gpsimd queue so output DMAs (sync)
                # don't queue behind
                cl = sv0
                for step in (8, 20, 20, 20, 20):
                    ch = min(cl + step, sv1)
                    if ch > cl:
                        nc.gpsimd.dma_start(
                            out=Xv[0][:, cl:ch, 0:JX],
                            in_=x_t[:, jy0 + cl:jy0 + ch, :])
                    cl = ch
                nc.gpsimd.memset(X4[:, 0:GL], 0.0)
                nc.gpsimd.memset(X4[:, GL + LXJ * RP:], 0.0)
                nc.gpsimd.memset(Xv[0][:, :, JX:RP], 0.0)
                if sv0 > 0:
                    nc.gpsimd.memset(Xv[0][:, 0:sv0, 0:JX], 0.0)
                if sv1 < LXJ:
                    nc.gpsimd.memset(Xv[0][:, sv1:LXJ, 0:JX], 0.0)

            def emit_mm(psum, src_v, sp, kind, r0, start, stop):
                k0, k1 = sp["k0"], sp["k1"]
                if kind == "c0":
                    rhs = src_v[sp["vx"]][32 * k0:32 * k1,
                                          r0 + sp["vy"]:r0 + sp["vy"] + 2,
                                          0:JX]
                else:
                    rhs = src_v[sp["vx"]][32 * k0:32 * k1,
                                          r0 + sp["vy"]:r0 + sp["vy"] + 7:2,
                                          0:JX - 1:2]
                lhsT = wtile[32 * k0:32 * k1, sp["off"]:sp["off"] + sp["wid"]]
                nc.tensor.matmul(
                    psum[32 * sp["m0"]:32 * sp["m1"], :], lhsT, rhs,
                    start=start, stop=stop,
                    tile_position=(32 * k0, 32 * sp["m0"]),
                    skip_group_check=True)

            def emit_conv0(t):
                X4, Xv = slabs[t]
                H0 = hpool.tile([128, TSH], F16, tag="H0")
                Hv = {d: H0[:, GL + d:GL + d + LHJ * RP].rearrange(
                    "p (s u) -> p s u", u=RP) for d in (-1, 0, 1, 2)}
                nc.gpsimd.memset(H0[:, 0:GL], 0.0)
                nc.gpsimd.memset(H0[:, GL + LHJ * RP:], 0.0)
                nc.gpsimd.memset(Hv[0][:, :, JX:RP], 0.0)
                jh0 = SO * t - 1
                if t == 0:
                    nc.gpsimd.memset(Hv[0][:, 0:1, 0:JX], 0.0)
                if t == T - 1:
                    nc.gpsimd.memset(Hv[0][:, LHJ - 3:LHJ - 2, 0:JX], 0.0)
                # slot-major interleave over group pairs: adjacent slots in
                # PE program order have the same shape (full/pair/quad);
                # alternate direction per block so shapes match across blocks
                c0slots = [[0], [1, 2], [3, 4, 5, 6], [7, 8, 9, 10]]
                g0 = 0
                blk = 0
                while g0 < NG0:
                    gs = [g for g in (g0, g0 + 1) if g < NG0]
                    slots = c0slots if blk % 2 == 0 else c0slots[::-1]
                    blk += 1
                    pss = {}
                    for g in gs:
                        psg = ppool0.tile([128, 512], F32, tag="ps0")
                        pss[g] = psg
                    for sl, slot in enumerate(slots):
                        for g in gs:
                            for si in slot:
                                emit_mm(pss[g], Xv, _SPECS["c0"][si],
                                        "c0", 2 * g + 1,
                                        sl == 0, sl == len(slots) - 1)
                    for g in gs:
                        w0v = max(0, -(jh0 + 2 * g))
                        w1v = min(2, NJY - (jh0 + 2 * g))
                        if w1v <= w0v:
                            continue
                        ps0v = pss[g][:, :].rearrange("p (a b) -> p a b", b=JX)
                        nc.scalar.activation(
                            Hv[0][:, 2 * g + w0v:2 * g + w1v, 0:JX],
                            ps0v[:, w0v:w1v, :],
                            ACTF.Prelu, bias=btile[:, 0:1], alpha=0.2)
                    g0 += 2
                return Hv

            def emit_c1rs(t, Hv):
                X4, Xv = slabs[t]
                qslots = [[0], [1], [2], [3], [4, 5], [6, 7],
                          [8, 9, 10, 11], [12, 13, 14, 15], [16, 17, 18, 19]]
                for gq in range(NGQ):
                    ps1 = ppool.tile([128, 512], F32, tag="ps1")
                    ps2 = ppool.tile([128, 512], F32, tag="ps2")
                    # like-shaped slots adjacent (fulls, pairs, quads), with
                    # direction alternating per gq to match across groups
                    phases = ((0, 4), (4, 6), (6, 9))
                    seq = []
                    for lo, hi in phases:
                        for sl in range(lo, hi):
                            seq.append((ps1, Hv, "c1", 8 * gq + 1, sl))
                        for sl in range(lo, hi):
                            seq.append((ps2, Xv, "rs", 8 * gq + 2, sl))
                    if gq % 2 == 1:
                        seq = seq[::-1]
                    fs = {}
                    for pos, (ps, src, cv, r0, sl) in enumerate(seq):
                        first = id(ps) not in fs
                        fs[id(ps)] = True
                        last = all(id(ps) != id(q[0]) for q in seq[pos + 1:])
                        for si in qslots[sl]:
                            emit_mm(ps, src, _SPECS[cv][si], cv, r0,
                                    first, last)
                    h1sb = gpool.tile([128, 512], F32, tag="h1sb")
                    nc.scalar.activation(h1sb[:, :], ps1[:, :], ACTF.Prelu,
                                         bias=btile[:, 1:2], alpha=0.2)
                    osum = gpool.tile([128, 512], F32, tag="osum")
                    nc.vector.tensor_add(osum[:, :], h1sb[:, :], ps2[:, :])
                    jq0 = (SO // 2) * t + 4 * gq
                    nc.sync.dma_start(
                        out=o_t[:, jq0:jq0 + 4, :],
                        in_=osum[:, :].rearrange("p (s u) -> p s u", u=128))

            emit_load(0)
            for t in range(T):
                if t + 1 < T:
                    emit_load(t + 1)
                Hv = emit_conv0(t)
                emit_c1rs(t, Hv)
                del slabs[t]

    nc.compile()
    return nc


_CACHE = {}
LAST_RESULTS = None


def _get_nc():
    if "nc" not in _CACHE:
        _CACHE["nc"] = _build()
    return _CACHE["nc"]


def kernel(x, w0, b0, w1, b1, w_res):
    from concourse.bass_utils import run_bass_kernel_spmd
    x = np.asarray(x, np.float32)
    wts, biases = _pack_host(np.asarray(w0), np.asarray(b0), np.asarray(w1),
                             np.asarray(b1), np.asarray(w_res))
    # host relayout: [N, y, x, c] -> per core [32*(2*(y%2)+(x%2))+c, jy, jx]
    xr = (x.reshape(N_CORES, H // 2, 2, W // 2, 2, C)
          .transpose(0, 2, 4, 5, 1, 3)
          .reshape(N_CORES, 128, H // 2, W // 2)
          .astype(np.float16))
    nc = _get_nc()
    in_maps = [{"x": np.ascontiguousarray(xr[i]), "wts": wts,
                "biases": biases} for i in range(N_CORES)]
    res = run_bass_kernel_spmd(nc, in_maps, core_ids=list(range(N_CORES)))
    global LAST_RESULTS
    LAST_RESULTS = res
    o = np.stack([res.results[i]["out"] for i in range(N_CORES)])
    # [N, 32*(2*qy+qx)+f, jy, jx] -> [N, 2*jy+qy, 2*jx+qx, f]
    HQ = H // 4
    out = (o.reshape(N_CORES, 2, 2, F, HQ, HQ)
           .transpose(0, 4, 1, 5, 2, 3)
           .reshape(N_CORES, H // 2, W // 2, F))
    return np.ascontiguousarray(out).astype(np.float32)


if __name__ == "__main__":
    _selftest()
    print("selftest ok, wcols =", _WCOLS)


# revision 5
# speedup vs baseline: 1.0024x; 1.0024x over previous
"""Trainium2 Bass kernel for nn_DiscriminatorBlock (StyleGAN2 discriminator
block), parity-class ("mod-2 x mod-2") layout.

Per core (batch 8 over 8 cores): all tensors live in SBUF as
  T[32*g + c, jy, jx]  with g = 2*(y%2) + (x%2), jy=y>>1, jx=x>>1
so a 3x3 (or fused 6x6/4x4 stride-2) conv becomes a small set of dense
matmuls: for each "view" (a jy/jx shift of the rhs), every 32x32 block
(g, m) of the 128x128 weight matrix carries one conv tap. The dense
view (0,0) carries 16 taps in one full-array matmul; edge/corner views
carry the wrap-around taps and are emitted as narrow tile-positioned
matmuls that can run concurrently on disjoint PE sub-arrays.

conv0 (3x3 SAME): 11 matmuls in 4 PE "slots" (full / edge-pair / two
4-way col-disjoint quads) per 2-row group; conv1 (FIR blur + 3x3
stride 2 == fused 6x6 stride 2): 20 matmuls in 9 slots per 8-out-row
group with fully dense K=128; residual (fused 4x4 stride 2): same
structure. Slots are emitted slot-major over group pairs with
direction alternating per block, so adjacent PE instructions keep the
same shape and col-disjoint members stream concurrently.

The x relayout to the class layout (and the inverse for the output) is
done host-side in numpy; the device only runs big contiguous DMAs.
FIR blurs + equalized-lr scales + sqrt(2)/sqrt(0.5) gains folded into
the fp16 weights on the host; fp32 PSUM accumulate; Prelu on ACT.
Measured: 309851 ns HW exec (8 cores), rel err 3.1e-4 (baseline was
646879 ns).
"""
import sys
import os
sys.path.insert(0, '/opt/trn_rl_repo')
import numpy as np

H, W, C, F = 512, 512, 32, 32
N_CORES = 8
JX = W // 2            # 256 jx values per row
RP = JX + 32           # row pitch with zero guard cols
GL = 32                # leading guard cols
SO = 64                # out1 rows per tile
RB = 8                 # jy rows per load chunk

SQ2 = float(np.sqrt(2.0))
SQH = float(np.sqrt(0.5))

SPLIT_X = True         # split x-edge matmuls for 4-way col concurrency


def _mk_specs():
    """Per-conv ordered matmul specs: view shift (vy,vx), k/m 32-block
    ranges, start/stop flags, weight-tile column offset."""
    specs = {"c0": [], "c1": [], "rs": []}
    off = [0]

    def add(conv, vy, vx, k0, k1, m0, m1, start=False, stop=False):
        w = 32 * (m1 - m0)
        specs[conv].append(dict(vy=vy, vx=vx, k0=k0, k1=k1, m0=m0, m1=m1,
                                start=start, stop=stop, off=off[0], wid=w))
        off[0] += w

    # ---- c0: 3x3 SAME, views (ey,ex) in {-1,0,1}^2 ----
    # 4 slots: full | edge-y pair | x-edge quad | corner quad
    add("c0", 0, 0, 0, 4, 0, 4, start=True)
    add("c0", -1, 0, 2, 4, 0, 2)
    add("c0", 1, 0, 0, 2, 2, 4)
    add("c0", 0, -1, 0, 4, 0, 1)
    add("c0", 0, 1, 0, 3, 1, 2)
    add("c0", 0, -1, 0, 4, 2, 3)
    add("c0", 0, 1, 0, 3, 3, 4)
    add("c0", -1, -1, 3, 4, 0, 1, stop=True)
    add("c0", -1, 1, 2, 3, 1, 2, stop=True)
    add("c0", 1, -1, 1, 2, 2, 3, stop=True)
    add("c0", 1, 1, 0, 1, 3, 4, stop=True)

    # ---- c1: fused 6x6 stride 2, views (ay,ax) in {-1,0,1,2}^2 ----
    add("c1", 0, 0, 0, 4, 0, 4, start=True)
    add("c1", 0, 1, 0, 4, 0, 4)
    add("c1", 1, 0, 0, 4, 0, 4)
    add("c1", 1, 1, 0, 4, 0, 4)
    add("c1", -1, 0, 0, 4, 0, 2)
    add("c1", 2, 0, 0, 4, 2, 4)
    add("c1", -1, 1, 0, 4, 0, 2)
    add("c1", 2, 1, 0, 4, 2, 4)
    if SPLIT_X:
        for vy in (0, 1):
            add("c1", vy, -1, 0, 4, 0, 1)
            add("c1", vy, -1, 0, 4, 2, 3)
            add("c1", vy, 2, 0, 4, 1, 2)
            add("c1", vy, 2, 0, 4, 3, 4)
    else:
        for vy in (0, 1):
            add("c1", vy, -1, 0, 4, 0, 3)
            add("c1", vy, 2, 0, 4, 1, 4)
    add("c1", -1, -1, 0, 4, 0, 1, stop=True)
    add("c1", -1, 2, 0, 4, 1, 2, stop=True)
    add("c1", 2, -1, 0, 4, 2, 3, stop=True)
    add("c1", 2, 2, 0, 4, 3, 4, stop=True)

    # ---- rs: fused 4x4 stride 2 ----
    add("rs", 0, 0, 0, 4, 0, 4, start=True)
    add("rs", 0, 1, 0, 4, 0, 4)
    add("rs", 1, 0, 0, 4, 0, 4)
    add("rs", 1, 1, 0, 4, 0, 4)
    add("rs", -1, 0, 2, 4, 0, 2)
    add("rs", 2, 0, 0, 2, 2, 4)
    add("rs", -1, 1, 2, 4, 0, 2)
    add("rs", 2, 1, 0, 2, 2, 4)
    if SPLIT_X:
        for vy in (0, 1):
            add("rs", vy, -1, 0, 4, 0, 1)
            add("rs", vy, -1, 0, 4, 2, 3)
            add("rs", vy, 2, 0, 3, 1, 2)
            add("rs", vy, 2, 0, 3, 3, 4)
    else:
        for vy in (0, 1):
            add("rs", vy, -1, 0, 4, 0, 3)
            add("rs", vy, 2, 0, 3, 1, 4)
    add("rs", -1, -1, 3, 4, 0, 1, stop=True)
    add("rs", -1, 2, 2, 3, 1, 2, stop=True)
    add("rs", 2, -1, 1, 2, 2, 3, stop=True)
    add("rs", 2, 2, 0, 1, 3, 4, stop=True)

    return specs, off[0]


_SPECS, _WCOLS = _mk_specs()


def _tap_c0(vy, vx, g, m):
    py, px = g >> 1, g & 1
    my, mx = m >> 1, m & 1
    dy = 2 * vy + py - my
    dx = 2 * vx + px - mx
    if -1 <= dy <= 1 and -1 <= dx <= 1:
        return (dy + 1, dx + 1)
    return None


def _tap_c1(vy, vx, g, m):
    py, px = g >> 1, g & 1
    qy, qx = m >> 1, m & 1
    ey, ex = vy - qy + 1, vx - qx + 1
    if 0 <= ey <= 2 and 0 <= ex <= 2:
        return (2 * ey + py, 2 * ex + px)
    return None


def _rs1(v, p, q):
    if p == 1:
        if v == q - 1:
            return 0
        if v == q:
            return 2
    else:
        if v == q:
            return 1
        if v == q + 1:
            return 3
    return None


def _tap_rs(vy, vx, g, m):
    py, px = g >> 1, g & 1
    qy, qx = m >> 1, m & 1
    dy = _rs1(vy, py, qy)
    dx = _rs1(vx, px, qx)
    if dy is None or dx is None:
        return None
    return (dy, dx)


_TAP = {"c0": _tap_c0, "c1": _tap_c1, "rs": _tap_rs}
_NTAP = {"c0": 3, "c1": 6, "rs": 4}


def _selftest():
    """Every (out-class, tap) pair must be covered exactly once."""
    for conv in ("c0", "c1", "rs"):
        n = _NTAP[conv]
        cnt = np.zeros((4, n, n), np.int32)
        for sp in _SPECS[conv]:
            for g in range(sp["k0"], sp["k1"]):
                for m in range(sp["m0"], sp["m1"]):
                    t = _TAP[conv](sp["vy"], sp["vx"], g, m)
                    if t is not None:
                        cnt[m, t[0], t[1]] += 1
        assert (cnt == 1).all(), (conv, cnt)
    return True


def _pack_host(w0, b0, w1, b1, w_res):
    bk = np.array([1.0, 3.0, 3.0, 1.0]) / 8.0
    W0e = w0.astype(np.float64) * (1.0 / np.sqrt(9 * 32))
    W1e = w1.astype(np.float64) * (1.0 / np.sqrt(9 * 32))
    WRe = w_res.astype(np.float64) * (1.0 / np.sqrt(1 * 32))
    K6 = np.zeros((6, 6, 32, 32))
    for r in range(3):
        for s in range(3):
            for a in range(4):
                for c in range(4):
                    K6[r + a, s + c] += bk[a] * bk[c] * W1e[r, s]
    K6 *= SQ2
    K4 = np.einsum('a,c,ij->acij', bk, bk, WRe[0, 0]) * SQH
    KK = {"c0": W0e, "c1": K6, "rs": K4}

    wts = np.zeros((128, _WCOLS), np.float16)
    for conv in ("c0", "c1", "rs"):
        for sp in _SPECS[conv]:
            for g in range(sp["k0"], sp["k1"]):
                for m in range(sp["m0"], sp["m1"]):
                    t = _TAP[conv](sp["vy"], sp["vx"], g, m)
                    if t is None:
                        continue
                    wts[32 * g:32 * g + 32,
                        sp["off"] + 32 * (m - sp["m0"]):
                        sp["off"] + 32 * (m - sp["m0"]) + 32] = \
                        KK[conv][t[0], t[1]].astype(np.float16)
    biases = np.zeros((128, 2), np.float32)
    biases[:, 0] = np.tile(b0.astype(np.float32).reshape(-1), 4)
    biases[:, 1] = np.tile(b1.astype(np.float32).reshape(-1), 4)
    return wts, biases


def _build():
    import concourse.mybir as mybir
    from concourse import bacc
    from concourse.tile import TileContext

    F32 = mybir.dt.float32
    F16 = mybir.dt.float16
    ACTF = mybir.ActivationFunctionType

    HO = H // 2            # 256 out1 rows
    T = HO // SO           # tiles
    NG0 = SO // 2 + 1      # c0 groups per tile (2 h-halfres rows each)
    NGQ = SO // 8          # c1/rs groups per tile (8 out rows each)
    LXJ = SO + 4           # x slab jy rows (68 used)
    LHJ = SO + 4           # h slab rows (66 used)
    NJY = H // 2           # 256 valid jy rows

    nc = bacc.Bacc('TRN2', target_bir_lowering=False)
    # x pre-relayouted on host: [32*g+c, jy, jx] fp16, g=2*(y%2)+(x%2)
    x_t = nc.dram_tensor("x", [128, NJY, JX], F16, kind="ExternalInput")
    w_t = nc.dram_tensor("wts", [128, _WCOLS], F16, kind="ExternalInput")
    b_t = nc.dram_tensor("biases", [128, 2], F32, kind="ExternalInput")
    # out in class layout: [32*m+f, jy'', jx''], un-shuffled on host
    o_t = nc.dram_tensor("out", [128, HO // 2, HO // 2], F32,
                         kind="ExternalOutput")

    TSX = GL + LXJ * RP + 8
    TSH = GL + LHJ * RP + 8

    with TileContext(nc) as tc:
        with tc.tile_pool(name="const", bufs=1) as cpool, \
             tc.tile_pool(name="slab", bufs=2) as spool, \
             tc.tile_pool(name="hslab", bufs=1) as hpool, \
             tc.tile_pool(name="stage", bufs=3) as gpool, \
             tc.tile_pool(name="psum0", bufs=4, space="PSUM") as ppool0, \
             tc.tile_pool(name="psum", bufs=2, space="PSUM") as ppool:

            wtile = cpool.tile([128, _WCOLS], F16, tag="wts")
            nc.sync.dma_start(out=wtile[:, :], in_=w_t[:, :])
            btile = cpool.tile([128, 2], F32, tag="bias")
            nc.sync.dma_start(out=btile[:, :], in_=b_t[:, :])

            slabs = {}

            def emit_load(t):
                jy0 = SO * t - 2
                X4 = spool.tile([128, TSX], F16, tag="X4")
                Xv = {d: X4[:, GL + d:GL + d + LXJ * RP].rearrange(
                    "p (s u) -> p s u", u=RP) for d in (-1, 0, 1, 2)}
                slabs[t] = (X4, Xv)

                sv0 = max(0, -jy0)
                sv1 = min(LXJ, NJY - jy0)
                # chunked (small first chunk) so the first conv0 groups can
                # start early; on the gpsimd queue so output DMAs (sync)
                # don't queue behind
                cl = sv0
                for step in (8, 20, 20, 20, 20):
                    ch = min(cl + step, sv1)
                    if ch > cl:
                        nc.gpsimd.dma_start(
                            out=Xv[0][:, cl:ch, 0:JX],
                            in_=x_t[:, jy0 + cl:jy0 + ch, :])
                    cl = ch
                nc.gpsimd.memset(X4[:, 0:GL], 0.0)
                nc.gpsimd.memset(X4[:, GL + LXJ * RP:], 0.0)
                nc.gpsimd.memset(Xv[0][:, :, JX:RP], 0.0)
                if sv0 > 0:
                    nc.gpsimd.memset(Xv[0][:, 0:sv0, 0:JX], 0.0)
                if sv1 < LXJ:
                    nc.gpsimd.memset(Xv[0][:, sv1:LXJ, 0:JX], 0.0)

            def emit_mm(psum, src_v, sp, kind, r0, start, stop):
                k0, k1 = sp["k0"], sp["k1"]
                if kind == "c0":
                    rhs = src_v[sp["vx"]][32 * k0:32 * k1,
                                          r0 + sp["vy"]:r0 + sp["vy"] + 2,
                                          0:JX]
                else:
                    rhs = src_v[sp["vx"]][32 * k0:32 * k1,
                                          r0 + sp["vy"]:r0 + sp["vy"] + 7:2,
                                          0:JX - 1:2]
                lhsT = wtile[32 * k0:32 * k1, sp["off"]:sp["off"] + sp["wid"]]
                nc.tensor.matmul(
                    psum[32 * sp["m0"]:32 * sp["m1"], :], lhsT, rhs,
                    start=start, stop=stop,
                    tile_position=(32 * k0, 32 * sp["m0"]),
                    skip_group_check=True)

            def emit_conv0(t):
                X4, Xv = slabs[t]
                H0 = hpool.tile([128, TSH], F16, tag="H0")
                Hv = {d: H0[:, GL + d:GL + d + LHJ * RP].rearrange(
                    "p (s u) -> p s u", u=RP) for d in (-1, 0, 1, 2)}
                nc.gpsimd.memset(H0[:, 0:GL], 0.0)
                nc.gpsimd.memset(H0[:, GL + LHJ * RP:], 0.0)
                nc.gpsimd.memset(Hv[0][:, :, JX:RP], 0.0)
                jh0 = SO * t - 1
                if t == 0:
                    nc.gpsimd.memset(Hv[0][:, 0:1, 0:JX], 0.0)
                if t == T - 1:
                    nc.gpsimd.memset(Hv[0][:, LHJ - 3:LHJ - 2, 0:JX], 0.0)
                # slot-major interleave over group pairs: adjacent slots in
                # PE program order have the same shape (full/pair/quad);
                # alternate direction per block so shapes match across blocks
                c0slots = [[0], [1, 2], [3, 4, 5, 6], [7, 8, 9, 10]]
                g0 = 0
                blk = 0
                while g0 < NG0:
                    gs = [g for g in (g0, g0 + 1) if g < NG0]
                    slots = c0slots if blk % 2 == 0 else c0slots[::-1]
                    blk += 1
                    pss = {}
                    for g in gs:
                        psg = ppool0.tile([128, 512], F32, tag="ps0")
                        pss[g] = psg
                    for sl, slot in enumerate(slots):
                        for g in gs:
                            for si in slot:
                                emit_mm(pss[g], Xv, _SPECS["c0"][si],
                                        "c0", 2 * g + 1,
                                        sl == 0, sl == len(slots) - 1)
                    for g in gs:
                        w0v = max(0, -(jh0 + 2 * g))
                        w1v = min(2, NJY - (jh0 + 2 * g))
                        if w1v <= w0v:
                            continue
                        ps0v = pss[g][:, :].rearrange("p (a b) -> p a b", b=JX)
                        nc.scalar.activation(
                            Hv[0][:, 2 * g + w0v:2 * g + w1v, 0:JX],
                            ps0v[:, w0v:w1v, :],
                            ACTF.Prelu, bias=btile[:, 0:1], alpha=0.2)
                    g0 += 2
                return Hv

            def emit_c1rs(t, Hv):
                X4, Xv = slabs[t]
                qslots = [[0], [1], [2], [3], [4, 5], [6, 7],
                          [8, 9, 10, 11], [12, 13, 14, 15], [16, 17, 18, 19]]
                for gq in range(NGQ):
                    ps1 = ppool.tile([128, 512], F32, tag="ps1")
                    ps2 = ppool.tile([128, 512], F32, tag="ps2")
                    # like-shaped slots adjacent (fulls, pairs, quads), with
                    # direction alternating per gq to match across groups
                    phases = ((0, 4), (4, 6), (6, 9))
                    seq = []
                    for lo, hi in phases:
                        for sl in range(lo, hi):
                            seq.append((ps1, Hv, "c1", 8 * gq + 1, sl))
                        for sl in range(lo, hi):
                            seq.append((ps2, Xv, "rs", 8 * gq + 2, sl))
                    if gq % 2 == 1:
                        seq = seq[::-1]
                    fs = {}
                    for pos, (ps, src, cv, r0, sl) in enumerate(seq):
                        first = id(ps) not in fs
                        fs[id(ps)] = True
                        last = all(id(ps) != id(q[0]) for q in seq[pos + 1:])
                        for si in qslots[sl]:
                            emit_mm(ps, src, _SPECS[cv][si], cv, r0,
                                    first, last)
                    h1sb = gpool.tile([128, 512], F32, tag="h1sb")
                    nc.scalar.activation(h1sb[:, :], ps1[:, :], ACTF.Prelu,
                                         bias=btile[:, 1:2], alpha=0.2)
                    osum = gpool.tile([128, 512], F32, tag="osum")
                    nc.vector.tensor_add(osum[:, :], h1sb[:, :], ps2[:, :])
                    jq0 = (SO // 2) * t + 4 * gq
                    nc.sync.dma_start(
                        out=o_t[:, jq0:jq0 + 4, :],
                        in_=osum[:, :].rearrange("p (s u) -> p s u", u=128))

            emit_load(0)
            for t in range(T):
                if t + 1 < T:
                    emit_load(t + 1)
                Hv = emit_conv0(t)
                emit_c1rs(t, Hv)
                del slabs[t]

    nc.compile()
    return nc


_CACHE = {}
LAST_RESULTS = None


def _get_nc():
    if "nc" not in _CACHE:
        _CACHE["nc"] = _build()
    return _CACHE["nc"]


def kernel(x, w0, b0, w1, b1, w_res):
    from concourse.bass_utils import run_bass_kernel_spmd
    x = np.asarray(x, np.float32)
    wts, biases = _pack_host(np.asarray(w0), np.asarray(b0), np.asarray(w1),
                             np.asarray(b1), np.asarray(w_res))
    # host relayout: [N, y, x, c] -> per core [32*(2*(y%2)+(x%2))+c, jy, jx]
    xr = (x.reshape(N_CORES, H // 2, 2, W // 2, 2, C)
          .transpose(0, 2, 4, 5, 1, 3)
          .reshape(N_CORES, 128, H // 2, W // 2)
          .astype(np.float16))
    nc = _get_nc()
    in_maps = [{"x": np.ascontiguousarray(xr[i]), "wts": wts,
                "biases": biases} for i in range(N_CORES)]
    res = run_bass_kernel_spmd(nc, in_maps, core_ids=list(range(N_CORES)))
    global LAST_RESULTS
    LAST_RESULTS = res
    o = np.stack([res.results[i]["out"] for i in range(N_CORES)])
    # [N, 32*(2*qy+qx)+f, jy, jx] -> [N, 2*jy+qy, 2*jx+qx, f]
    HQ = H // 4
    out = (o.reshape(N_CORES, 2, 2, F, HQ, HQ)
           .transpose(0, 4, 1, 5, 2, 3)
           .reshape(N_CORES, H // 2, W // 2, F))
    return np.ascontiguousarray(out).astype(np.float32)


if __name__ == "__main__":
    _selftest()
    print("selftest ok, wcols =", _WCOLS)


# revision 6
# speedup vs baseline: 1.0058x; 1.0033x over previous
"""Trainium2 Bass kernel for nn_DiscriminatorBlock (StyleGAN2 discriminator
block), parity-class ("mod-2 x mod-2") layout.

Per core (batch 8 over 8 cores): all tensors live in SBUF as
  T[32*g + c, jy, jx]  with g = 2*(y%2) + (x%2), jy=y>>1, jx=x>>1
so a 3x3 (or fused 6x6/4x4 stride-2) conv becomes a small set of dense
matmuls: for each "view" (a jy/jx shift of the rhs), every 32x32 block
(g, m) of the 128x128 weight matrix carries one conv tap. The dense
view (0,0) carries 16 taps in one full-array matmul; edge/corner views
carry the wrap-around taps and are emitted as narrow tile-positioned
matmuls that can run concurrently on disjoint PE sub-arrays.

conv0 (3x3 SAME): 13 matmuls in 4 PE "slots" (full + three 4-way
col-disjoint quads) per 2-row group; conv1 (FIR blur + 3x3 stride 2 ==
fused 6x6 stride 2): 24 matmuls in 9 slots per 8-out-row group with
fully dense K=128 (9 slots = per-column view count = provable floor);
residual (fused 4x4 stride 2): same structure. Slots are emitted
slot-major over group pairs with direction alternating per block so
adjacent PE instructions keep the same shape, and col-disjoint members
stream concurrently on the PE sub-arrays.

The x relayout into the class layout (and the inverse for the output)
is done host-side in numpy; the device only runs big contiguous DMAs
(input chunks on the gpsimd queue, outputs on sync, guard memsets on
the vector engine so nothing queues behind the loads). FIR blurs +
equalized-lr scales + sqrt(2)/sqrt(0.5) gains folded into the fp16
weights on the host; fp32 PSUM accumulate; Prelu on ACT.
Measured: 308001 ns HW exec (8 cores), rel err 3.1e-4; baseline was
646879 ns.
"""
import sys
import os
sys.path.insert(0, '/opt/trn_rl_repo')
import numpy as np

H, W, C, F = 512, 512, 32, 32
N_CORES = 8
JX = W // 2            # 256 jx values per row
RP = JX + 32           # row pitch with zero guard cols
GL = 32                # leading guard cols
SO = 64                # out1 rows per tile
RB = 8                 # jy rows per load chunk

SQ2 = float(np.sqrt(2.0))
SQH = float(np.sqrt(0.5))

SPLIT_X = True         # split x-edge matmuls for 4-way col concurrency


def _mk_specs():
    """Per-conv ordered matmul specs: view shift (vy,vx), k/m 32-block
    ranges, start/stop flags, weight-tile column offset."""
    specs = {"c0": [], "c1": [], "rs": []}
    off = [0]

    def add(conv, vy, vx, k0, k1, m0, m1, start=False, stop=False):
        w = 32 * (m1 - m0)
        specs[conv].append(dict(vy=vy, vx=vx, k0=k0, k1=k1, m0=m0, m1=m1,
                                start=start, stop=stop, off=off[0], wid=w))
        off[0] += w

    # ---- c0: 3x3 SAME, views (ey,ex) in {-1,0,1}^2 ----
    # 4 slots: full | edge-y pair | x-edge quad | corner quad
    add("c0", 0, 0, 0, 4, 0, 4, start=True)
    add("c0", -1, 0, 2, 4, 0, 1)
    add("c0", -1, 0, 2, 4, 1, 2)
    add("c0", 1, 0, 0, 2, 2, 3)
    add("c0", 1, 0, 0, 2, 3, 4)
    add("c0", 0, -1, 0, 4, 0, 1)
    add("c0", 0, 1, 0, 3, 1, 2)
    add("c0", 0, -1, 0, 4, 2, 3)
    add("c0", 0, 1, 0, 3, 3, 4)
    add("c0", -1, -1, 3, 4, 0, 1, stop=True)
    add("c0", -1, 1, 2, 3, 1, 2, stop=True)
    add("c0", 1, -1, 1, 2, 2, 3, stop=True)
    add("c0", 1, 1, 0, 1, 3, 4, stop=True)

    # ---- c1: fused 6x6 stride 2, views (ay,ax) in {-1,0,1,2}^2 ----
    add("c1", 0, 0, 0, 4, 0, 4, start=True)
    add("c1", 0, 1, 0, 4, 0, 4)
    add("c1", 1, 0, 0, 4, 0, 4)
    add("c1", 1, 1, 0, 4, 0, 4)
    add("c1", -1, 0, 0, 4, 0, 1)
    add("c1", -1, 0, 0, 4, 1, 2)
    add("c1", 2, 0, 0, 4, 2, 3)
    add("c1", 2, 0, 0, 4, 3, 4)
    add("c1", -1, 1, 0, 4, 0, 1)
    add("c1", -1, 1, 0, 4, 1, 2)
    add("c1", 2, 1, 0, 4, 2, 3)
    add("c1", 2, 1, 0, 4, 3, 4)
    if SPLIT_X:
        for vy in (0, 1):
            add("c1", vy, -1, 0, 4, 0, 1)
            add("c1", vy, -1, 0, 4, 2, 3)
            add("c1", vy, 2, 0, 4, 1, 2)
            add("c1", vy, 2, 0, 4, 3, 4)
    else:
        for vy in (0, 1):
            add("c1", vy, -1, 0, 4, 0, 3)
            add("c1", vy, 2, 0, 4, 1, 4)
    add("c1", -1, -1, 0, 4, 0, 1, stop=True)
    add("c1", -1, 2, 0, 4, 1, 2, stop=True)
    add("c1", 2, -1, 0, 4, 2, 3, stop=True)
    add("c1", 2, 2, 0, 4, 3, 4, stop=True)

    # ---- rs: fused 4x4 stride 2 ----
    add("rs", 0, 0, 0, 4, 0, 4, start=True)
    add("rs", 0, 1, 0, 4, 0, 4)
    add("rs", 1, 0, 0, 4, 0, 4)
    add("rs", 1, 1, 0, 4, 0, 4)
    add("rs", -1, 0, 2, 4, 0, 1)
    add("rs", -1, 0, 2, 4, 1, 2)
    add("rs", 2, 0, 0, 2, 2, 3)
    add("rs", 2, 0, 0, 2, 3, 4)
    add("rs", -1, 1, 2, 4, 0, 1)
    add("rs", -1, 1, 2, 4, 1, 2)
    add("rs", 2, 1, 0, 2, 2, 3)
    add("rs", 2, 1, 0, 2, 3, 4)
    if SPLIT_X:
        for vy in (0, 1):
            add("rs", vy, -1, 0, 4, 0, 1)
            add("rs", vy, -1, 0, 4, 2, 3)
            add("rs", vy, 2, 0, 3, 1, 2)
            add("rs", vy, 2, 0, 3, 3, 4)
    else:
        for vy in (0, 1):
            add("rs", vy, -1, 0, 4, 0, 3)
            add("rs", vy, 2, 0, 3, 1, 4)
    add("rs", -1, -1, 3, 4, 0, 1, stop=True)
    add("rs", -1, 2, 2, 3, 1, 2, stop=True)
    add("rs", 2, -1, 1, 2, 2, 3, stop=True)
    add("rs", 2, 2, 0, 1, 3, 4, stop=True)

    return specs, off[0]


_SPECS, _WCOLS = _mk_specs()


def _tap_c0(vy, vx, g, m):
    py, px = g >> 1, g & 1
    my, mx = m >> 1, m & 1
    dy = 2 * vy + py - my
    dx = 2 * vx + px - mx
    if -1 <= dy <= 1 and -1 <= dx <= 1:
        return (dy + 1, dx + 1)
    return None


def _tap_c1(vy, vx, g, m):
    py, px = g >> 1, g & 1
    qy, qx = m >> 1, m & 1
    ey, ex = vy - qy + 1, vx - qx + 1
    if 0 <= ey <= 2 and 0 <= ex <= 2:
        return (2 * ey + py, 2 * ex + px)
    return None


def _rs1(v, p, q):
    if p == 1:
        if v == q - 1:
            return 0
        if v == q:
            return 2
    else:
        if v == q:
            return 1
        if v == q + 1:
            return 3
    return None


def _tap_rs(vy, vx, g, m):
    py, px = g >> 1, g & 1
    qy, qx = m >> 1, m & 1
    dy = _rs1(vy, py, qy)
    dx = _rs1(vx, px, qx)
    if dy is None or dx is None:
        return None
    return (dy, dx)


_TAP = {"c0": _tap_c0, "c1": _tap_c1, "rs": _tap_rs}
_NTAP = {"c0": 3, "c1": 6, "rs": 4}


def _selftest():
    """Every (out-class, tap) pair must be covered exactly once."""
    for conv in ("c0", "c1", "rs"):
        n = _NTAP[conv]
        cnt = np.zeros((4, n, n), np.int32)
        for sp in _SPECS[conv]:
            for g in range(sp["k0"], sp["k1"]):
                for m in range(sp["m0"], sp["m1"]):
                    t = _TAP[conv](sp["vy"], sp["vx"], g, m)
                    if t is not None:
                        cnt[m, t[0], t[1]] += 1
        assert (cnt == 1).all(), (conv, cnt)
    return True


def _pack_host(w0, b0, w1, b1, w_res):
    bk = np.array([1.0, 3.0, 3.0, 1.0]) / 8.0
    W0e = w0.astype(np.float64) * (1.0 / np.sqrt(9 * 32))
    W1e = w1.astype(np.float64) * (1.0 / np.sqrt(9 * 32))
    WRe = w_res.astype(np.float64) * (1.0 / np.sqrt(1 * 32))
    K6 = np.zeros((6, 6, 32, 32))
    for r in range(3):
        for s in range(3):
            for a in range(4):
                for c in range(4):
                    K6[r + a, s + c] += bk[a] * bk[c] * W1e[r, s]
    K6 *= SQ2
    K4 = np.einsum('a,c,ij->acij', bk, bk, WRe[0, 0]) * SQH
    KK = {"c0": W0e, "c1": K6, "rs": K4}

    wts = np.zeros((128, _WCOLS), np.float16)
    for conv in ("c0", "c1", "rs"):
        for sp in _SPECS[conv]:
            for g in range(sp["k0"], sp["k1"]):
                for m in range(sp["m0"], sp["m1"]):
                    t = _TAP[conv](sp["vy"], sp["vx"], g, m)
                    if t is None:
                        continue
                    wts[32 * g:32 * g + 32,
                        sp["off"] + 32 * (m - sp["m0"]):
                        sp["off"] + 32 * (m - sp["m0"]) + 32] = \
                        KK[conv][t[0], t[1]].astype(np.float16)
    biases = np.zeros((128, 2), np.float32)
    biases[:, 0] = np.tile(b0.astype(np.float32).reshape(-1), 4)
    biases[:, 1] = np.tile(b1.astype(np.float32).reshape(-1), 4)
    return wts, biases


def _build():
    import concourse.mybir as mybir
    from concourse import bacc
    from concourse.tile import TileContext

    F32 = mybir.dt.float32
    F16 = mybir.dt.float16
    ACTF = mybir.ActivationFunctionType

    HO = H // 2            # 256 out1 rows
    T = HO // SO           # tiles
    NG0 = SO // 2 + 1      # c0 groups per tile (2 h-halfres rows each)
    NGQ = SO // 8          # c1/rs groups per tile (8 out rows each)
    LXJ = SO + 4           # x slab jy rows (68 used)
    LHJ = SO + 4           # h slab rows (66 used)
    NJY = H // 2           # 256 valid jy rows

    nc = bacc.Bacc('TRN2', target_bir_lowering=False)
    # x pre-relayouted on host: [32*g+c, jy, jx] fp16, g=2*(y%2)+(x%2)
    x_t = nc.dram_tensor("x", [128, NJY, JX], F16, kind="ExternalInput")
    w_t = nc.dram_tensor("wts", [128, _WCOLS], F16, kind="ExternalInput")
    b_t = nc.dram_tensor("biases", [128, 2], F32, kind="ExternalInput")
    # out in class layout: [32*m+f, jy'', jx''], un-shuffled on host
    o_t = nc.dram_tensor("out", [128, HO // 2, HO // 2], F32,
                         kind="ExternalOutput")

    TSX = GL + LXJ * RP + 8
    TSH = GL + LHJ * RP + 8

    with TileContext(nc) as tc:
        with tc.tile_pool(name="const", bufs=1) as cpool, \
             tc.tile_pool(name="slab", bufs=2) as spool, \
             tc.tile_pool(name="hslab", bufs=1) as hpool, \
             tc.tile_pool(name="stage", bufs=3) as gpool, \
             tc.tile_pool(name="psum0", bufs=4, space="PSUM") as ppool0, \
             tc.tile_pool(name="psum", bufs=2, space="PSUM") as ppool:

            wtile = cpool.tile([128, _WCOLS], F16, tag="wts")
            nc.sync.dma_start(out=wtile[:, :], in_=w_t[:, :])
            btile = cpool.tile([128, 2], F32, tag="bias")
            nc.sync.dma_start(out=btile[:, :], in_=b_t[:, :])

            slabs = {}

            def emit_load(t):
                jy0 = SO * t - 2
                X4 = spool.tile([128, TSX], F16, tag="X4")
                Xv = {d: X4[:, GL + d:GL + d + LXJ * RP].rearrange(
                    "p (s u) -> p s u", u=RP) for d in (-1, 0, 1, 2)}
                slabs[t] = (X4, Xv)

                sv0 = max(0, -jy0)
                sv1 = min(LXJ, NJY - jy0)
                # guard memsets on the vector engine so they overlap the
                # chunked loads (gpsimd) instead of queueing behind them;
                # small first chunk so the first conv0 groups start early
                nc.vector.memset(X4[:, 0:GL], 0.0)
                nc.vector.memset(X4[:, GL + LXJ * RP:], 0.0)
                nc.vector.memset(Xv[0][:, :, JX:RP], 0.0)
                if sv0 > 0:
                    nc.vector.memset(Xv[0][:, 0:sv0, 0:JX], 0.0)
                if sv1 < LXJ:
                    nc.vector.memset(Xv[0][:, sv1:LXJ, 0:JX], 0.0)
                cl = sv0
                for step in (8, 20, 20, 20, 20):
                    ch = min(cl + step, sv1)
                    if ch > cl:
                        nc.gpsimd.dma_start(
                            out=Xv[0][:, cl:ch, 0:JX],
                            in_=x_t[:, jy0 + cl:jy0 + ch, :])
                    cl = ch

            def emit_mm(psum, src_v, sp, kind, r0, start, stop):
                k0, k1 = sp["k0"], sp["k1"]
                if kind == "c0":
                    rhs = src_v[sp["vx"]][32 * k0:32 * k1,
                                          r0 + sp["vy"]:r0 + sp["vy"] + 2,
                                          0:JX]
                else:
                    rhs = src_v[sp["vx"]][32 * k0:32 * k1,
                                          r0 + sp["vy"]:r0 + sp["vy"] + 7:2,
                                          0:JX - 1:2]
                lhsT = wtile[32 * k0:32 * k1, sp["off"]:sp["off"] + sp["wid"]]
                nc.tensor.matmul(
                    psum[32 * sp["m0"]:32 * sp["m1"], :], lhsT, rhs,
                    start=start, stop=stop,
                    tile_position=(32 * k0, 32 * sp["m0"]),
                    skip_group_check=True)

            def emit_conv0(t):
                X4, Xv = slabs[t]
                H0 = hpool.tile([128, TSH], F16, tag="H0")
                Hv = {d: H0[:, GL + d:GL + d + LHJ * RP].rearrange(
                    "p (s u) -> p s u", u=RP) for d in (-1, 0, 1, 2)}
                nc.vector.memset(H0[:, 0:GL], 0.0)
                nc.vector.memset(H0[:, GL + LHJ * RP:], 0.0)
                nc.vector.memset(Hv[0][:, :, JX:RP], 0.0)
                jh0 = SO * t - 1
                if t == 0:
                    nc.vector.memset(Hv[0][:, 0:1, 0:JX], 0.0)
                if t == T - 1:
                    nc.vector.memset(Hv[0][:, LHJ - 3:LHJ - 2, 0:JX], 0.0)
                # slot-major interleave over group pairs: adjacent slots in
                # PE program order have the same shape (full/pair/quad);
                # alternate direction per block so shapes match across blocks
                c0slots = [[0], [1, 2, 3, 4], [5, 6, 7, 8], [9, 10, 11, 12]]
                g0 = 0
                blk = 0
                while g0 < NG0:
                    gs = [g for g in (g0, g0 + 1) if g < NG0]
                    slots = c0slots if blk % 2 == 0 else c0slots[::-1]
                    blk += 1
                    pss = {}
                    for g in gs:
                        psg = ppool0.tile([128, 512], F32, tag="ps0")
                        pss[g] = psg
                    for sl, slot in enumerate(slots):
                        for g in gs:
                            for si in slot:
                                emit_mm(pss[g], Xv, _SPECS["c0"][si],
                                        "c0", 2 * g + 1,
                                        sl == 0, sl == len(slots) - 1)
                    for g in gs:
                        w0v = max(0, -(jh0 + 2 * g))
                        w1v = min(2, NJY - (jh0 + 2 * g))
                        if w1v <= w0v:
                            continue
                        ps0v = pss[g][:, :].rearrange("p (a b) -> p a b", b=JX)
                        nc.scalar.activation(
                            Hv[0][:, 2 * g + w0v:2 * g + w1v, 0:JX],
                            ps0v[:, w0v:w1v, :],
                            ACTF.Prelu, bias=btile[:, 0:1], alpha=0.2)
                    g0 += 2
                return Hv

            def emit_c1rs(t, Hv):
                X4, Xv = slabs[t]
                qslots = [[0], [1], [2], [3], [4, 5, 6, 7], [8, 9, 10, 11],
                          [12, 13, 14, 15], [16, 17, 18, 19],
                          [20, 21, 22, 23]]
                for gq in range(NGQ):
                    ps1 = ppool.tile([128, 512], F32, tag="ps1")
                    ps2 = ppool.tile([128, 512], F32, tag="ps2")
                    # like-shaped slots adjacent (fulls, pairs, quads), with
                    # direction alternating per gq to match across groups
                    phases = ((0, 4), (4, 9))
                    seq = []
                    for lo, hi in phases:
                        for sl in range(lo, hi):
                            seq.append((ps1, Hv, "c1", 8 * gq + 1, sl))
                        for sl in range(lo, hi):
                            seq.append((ps2, Xv, "rs", 8 * gq + 2, sl))
                    if gq % 2 == 1:
                        seq = seq[::-1]
                    fs = {}
                    for pos, (ps, src, cv, r0, sl) in enumerate(seq):
                        first = id(ps) not in fs
                        fs[id(ps)] = True
                        last = all(id(ps) != id(q[0]) for q in seq[pos + 1:])
                        for si in qslots[sl]:
                            emit_mm(ps, src, _SPECS[cv][si], cv, r0,
                                    first, last)
                    h1sb = gpool.tile([128, 512], F32, tag="h1sb")
                    nc.scalar.activation(h1sb[:, :], ps1[:, :], ACTF.Prelu,
                                         bias=btile[:, 1:2], alpha=0.2)
                    osum = gpool.tile([128, 512], F32, tag="osum")
                    nc.vector.tensor_add(osum[:, :], h1sb[:, :], ps2[:, :])
                    jq0 = (SO // 2) * t + 4 * gq
                    nc.sync.dma_start(
                        out=o_t[:, jq0:jq0 + 4, :],
                        in_=osum[:, :].rearrange("p (s u) -> p s u", u=128))

            emit_load(0)
            for t in range(T):
                if t + 1 < T:
                    emit_load(t + 1)
                Hv = emit_conv0(t)
                emit_c1rs(t, Hv)
                del slabs[t]

    nc.compile()
    return nc


_CACHE = {}
LAST_RESULTS = None


def _get_nc():
    if "nc" not in _CACHE:
        _CACHE["nc"] = _build()
    return _CACHE["nc"]


def kernel(x, w0, b0, w1, b1, w_res):
    from concourse.bass_utils import run_bass_kernel_spmd
    x = np.asarray(x, np.float32)
    wts, biases = _pack_host(np.asarray(w0), np.asarray(b0), np.asarray(w1),
                             np.asarray(b1), np.asarray(w_res))
    # host relayout: [N, y, x, c] -> per core [32*(2*(y%2)+(x%2))+c, jy, jx]
    xr = (x.reshape(N_CORES, H // 2, 2, W // 2, 2, C)
          .transpose(0, 2, 4, 5, 1, 3)
          .reshape(N_CORES, 128, H // 2, W // 2)
          .astype(np.float16))
    nc = _get_nc()
    in_maps = [{"x": np.ascontiguousarray(xr[i]), "wts": wts,
                "biases": biases} for i in range(N_CORES)]
    res = run_bass_kernel_spmd(nc, in_maps, core_ids=list(range(N_CORES)))
    global LAST_RESULTS
    LAST_RESULTS = res
    o = np.stack([res.results[i]["out"] for i in range(N_CORES)])
    # [N, 32*(2*qy+qx)+f, jy, jx] -> [N, 2*jy+qy, 2*jx+qx, f]
    HQ = H // 4
    out = (o.reshape(N_CORES, 2, 2, F, HQ, HQ)
           .transpose(0, 4, 1, 5, 2, 3)
           .reshape(N_CORES, H // 2, W // 2, F))
    return np.ascontiguousarray(out).astype(np.float32)


if __name__ == "__main__":
    _selftest()
    print("selftest ok, wcols =", _WCOLS)
